# revision 7
# baseline (speedup 1.0000x reference)
"""Trainium2 Bass kernel for nn_LookupTablePosMy (embedding lookups + LSTM + windowed dot-product head).

Strategy: pure data-parallel over the batch (4096 -> 512 rows/core on 8 cores), no collectives.
Per core:
  - gather input1 embeddings (indirect DMA), PE-transpose to [emb, batch] bf16
  - LSTM over 19 steps: gates.T [7680, 512] computed in 60 [128,512] PSUM tiles,
    each accumulating 1 emb-projection chunk + 15 hidden chunks (bf16 matmuls),
    W_hh streamed from HBM each step in host-pre-transposed tile order,
    pos_table@W_ih.T + biases folded into the ACT bias operand.
  - head: gather input2 embeddings, transpose final h, 90 sliding-window dot
    products per sample via tensor_tensor_reduce (mult + chained max),
    log-softmax of 2 classes via Softplus.
"""

import sys

for _p in ("/opt/trn_rl_repo", "/opt/pypackages"):
    if _p not in sys.path:
        sys.path.append(_p)

import numpy as np
import ml_dtypes
from contextlib import ExitStack

import concourse.bass as bass
import concourse.bacc as bacc
import concourse.tile as tile
from concourse import mybir
from concourse.bass import IndirectOffsetOnAxis
from concourse.bass_utils import run_bass_kernel_spmd
from concourse.masks import make_identity

AF = mybir.ActivationFunctionType
ALU = mybir.AluOpType
F32 = mybir.dt.float32
BF16 = mybir.dt.bfloat16
I32 = mybir.dt.int32

NCORES = 8
B = 4096
BL = B // NCORES          # 512 batch rows per core
P = 128
SEQ = 19
E = 128
H = 1920                  # hidden size
KT = H // P               # 15 hidden k-tiles
G = 4 * H                 # 7680 gate rows
MT = G // P               # 60 gate m-tiles
N2 = 20
NW = 18                   # windows
K5 = 5
D3 = 384                  # window dot width
JB = BL // P              # 4 batch tiles per core

TRACE = False             # test.py sets this for profiling runs
LAST_RESULTS = None       # BassKernelResults of last run (for test.py)

_COMPILED = None          # cached (nc,) built program


def _build_program(seq=SEQ, with_head=True, with_gathers=True):
    nc = bacc.Bacc("TRN2", target_bir_lowering=False, debug=False,
                   enable_asserts=False, num_devices=NCORES)

    # DRAM I/O (per-core shapes)
    idx1 = nc.dram_tensor("idx1", [P, JB * SEQ], I32, kind="ExternalInput").ap()
    idx2 = nc.dram_tensor("idx2", [P, JB * N2], I32, kind="ExternalInput").ap()
    emb = nc.dram_tensor("emb", [100000, E], F32, kind="ExternalInput").ap()
    whh = nc.dram_tensor("whh", [KT, P, 4 * KT * P], BF16, kind="ExternalInput").ap()
    wie = nc.dram_tensor("wie", [P, MT * P], BF16, kind="ExternalInput").ap()
    pp = nc.dram_tensor("pp", [P, SEQ * MT], F32, kind="ExternalInput").ap()
    lin = nc.dram_tensor("lin", [P, 4], F32, kind="ExternalInput").ap()
    out = nc.dram_tensor("out", [BL, 2], F32, kind="ExternalOutput").ap()

    with tile.TileContext(nc) as tc, ExitStack() as ctx:
        const_pool = ctx.enter_context(tc.tile_pool(name="const", bufs=1))
        w_pool = ctx.enter_context(tc.tile_pool(name="w", bufs=2))
        gather_pool = ctx.enter_context(tc.tile_pool(name="gat", bufs=12))
        gcast_pool = ctx.enter_context(tc.tile_pool(name="gc", bufs=8))
        e1t_pool = ctx.enter_context(tc.tile_pool(name="e1t", bufs=8))
        h_pool = ctx.enter_context(tc.tile_pool(name="h", bufs=31))
        c_pool = ctx.enter_context(tc.tile_pool(name="c", bufs=KT))
        gact_pool = ctx.enter_context(tc.tile_pool(name="gact", bufs=8))
        emb2_pool = ctx.enter_context(tc.tile_pool(name="emb2", bufs=JB))
        hT_pool = ctx.enter_context(tc.tile_pool(name="hT", bufs=JB))
        head_pool = ctx.enter_context(tc.tile_pool(name="hsc", bufs=4))
        small_pool = ctx.enter_context(tc.tile_pool(name="small", bufs=16))
        mm_psum = ctx.enter_context(tc.tile_pool(name="mmps", bufs=6, space="PSUM"))
        tr_psum = ctx.enter_context(tc.tile_pool(name="trps", bufs=2, space="PSUM"))

        # constants
        ident = const_pool.tile([P, P], BF16)
        make_identity(nc, ident[:])
        idx1_sb = const_pool.tile([P, JB * SEQ], I32)
        nc.sync.dma_start(idx1_sb[:], idx1[:])
        idx2_sb = const_pool.tile([P, JB * N2], I32)
        nc.sync.dma_start(idx2_sb[:], idx2[:])
        pp_sb = const_pool.tile([P, SEQ * MT], F32)
        nc.sync.dma_start(pp_sb[:], pp[:])
        wie_sb = const_pool.tile([P, MT * P], BF16)
        nc.sync.dma_start(wie_sb[:], wie[:])
        lin_sb = const_pool.tile([P, 4], F32)
        nc.sync.dma_start(lin_sb[:], lin[:])

        # ---- input1 embedding gathers -> transpose -> e1t[t] [128(emb), 512(batch)] bf16
        e1t = []
        for t in range(seq):
            et = e1t_pool.tile([P, BL], BF16, tag="e1t")
            for j in range(JB):
                if with_gathers:
                    gt = gather_pool.tile([P, E], F32, tag="gat")
                    nc.gpsimd.indirect_dma_start(
                        out=gt[:], out_offset=None, in_=emb[:],
                        in_offset=IndirectOffsetOnAxis(
                            ap=idx1_sb[:, j * SEQ + t: j * SEQ + t + 1], axis=0))
                    gc = gcast_pool.tile([P, E], BF16, tag="gc")
                    nc.vector.tensor_copy(gc[:], gt[:])
                    tp = tr_psum.tile([P, P], BF16, tag="trps")
                    nc.tensor.transpose(tp[:], gc[:], ident[:])
                    nc.vector.tensor_copy(et[:, j * P:(j + 1) * P], tp[:])
                else:
                    nc.vector.memset(et[:, j * P:(j + 1) * P], 0.01)
            e1t.append(et)

        # ---- input2 embedding gathers -> emb2[j] [128(batch), 2560(emb)] bf16
        emb2 = []
        for j in range(JB):
            e2 = emb2_pool.tile([P, N2 * E], BF16, tag="emb2")
            for n in range(N2):
                if with_gathers and with_head:
                    gt = gather_pool.tile([P, E], F32, tag="gat")
                    nc.gpsimd.indirect_dma_start(
                        out=gt[:], out_offset=None, in_=emb[:],
                        in_offset=IndirectOffsetOnAxis(
                            ap=idx2_sb[:, j * N2 + n: j * N2 + n + 1], axis=0))
                    nc.vector.tensor_copy(e2[:, n * E:(n + 1) * E], gt[:])
                else:
                    nc.vector.memset(e2[:, n * E:(n + 1) * E], 0.01)
            emb2.append(e2)

        # ---- LSTM over 19 steps
        c_tiles = [c_pool.tile([P, BL], F32, tag="c", name=f"c{_k}") for _k in range(KT)]
        h_prev = None
        for s in range(seq):
            h_new = []
            for t in range(KT):
                if s > 0:
                    wt = w_pool.tile([P, 4 * KT * P], BF16, tag="w")
                    nc.sync.dma_start(wt[:], whh[t])
                acts = []
                for q in range(4):
                    m = q * KT + t
                    ps = mm_psum.tile([P, BL], F32, tag="mmps")
                    nc.tensor.matmul(ps[:], wie_sb[:, m * P:(m + 1) * P], e1t[s][:],
                                     start=True, stop=(s == 0))
                    if s > 0:
                        for kk in range(KT):
                            nc.tensor.matmul(
                                ps[:],
                                wt[:, (q * KT + kk) * P:(q * KT + kk + 1) * P],
                                h_prev[kk][:],
                                start=False, stop=(kk == KT - 1))
                    func = AF.Tanh if q == 2 else AF.Sigmoid
                    dst = gact_pool.tile([P, BL], F32, tag="gact")
                    nc.scalar.activation(dst[:], ps[:], func,
                                         bias=pp_sb[:, s * MT + m: s * MT + m + 1])
                    acts.append(dst)
                i_, f_, g_, o_ = acts
                ct = c_tiles[t]
                if s == 0:
                    nc.vector.tensor_tensor(ct[:], i_[:], g_[:], op=ALU.mult)
                else:
                    nc.vector.tensor_tensor(i_[:], i_[:], g_[:], op=ALU.mult)
                    nc.vector.tensor_tensor(ct[:], f_[:], ct[:], op=ALU.mult)
                    nc.vector.tensor_tensor(ct[:], ct[:], i_[:], op=ALU.add)
                nc.scalar.activation(g_[:], ct[:], AF.Tanh)
                hn = h_pool.tile([P, BL], BF16, tag="h")
                nc.vector.tensor_tensor(hn[:], o_[:], g_[:], op=ALU.mult)
                h_new.append(hn)
            h_prev = h_new

        # ---- head
        for j in range(JB):
            hT = hT_pool.tile([P, H], BF16, tag="hT")
            for k in range(KT):
                tp = tr_psum.tile([P, P], BF16, tag="trps")
                nc.tensor.transpose(tp[:], h_prev[k][:, j * P:(j + 1) * P], ident[:])
                nc.vector.tensor_copy(hT[:, k * P:(k + 1) * P], tp[:])
            ms = small_pool.tile([P, 1], F32, tag="ms")
            rs = small_pool.tile([P, K5 * NW], F32, tag="rs")
            for n in range(NW):
                win = emb2[j][:, n * E:n * E + D3]
                win3 = win.rearrange("p (o d) -> p o d", o=1).to_broadcast(
                    [P, K5, D3])
                prod = head_pool.tile([P, H], BF16, tag="hsc")
                nc.vector.tensor_tensor(prod[:], hT[:], win3, op=ALU.mult)
                nc.vector.tensor_reduce(
                    out=rs[:, n * K5:(n + 1) * K5],
                    in_=prod[:].rearrange("p (k d) -> p k d", k=K5),
                    axis=mybir.AxisListType.X, op=ALU.add)
            nc.vector.tensor_reduce(out=ms[:, 0:1], in_=rs[:],
                                    axis=mybir.AxisListType.X, op=ALU.max)
            # logits = log_softmax([w0*ms+b0, w1*ms+b1]):
            #   l0 = -softplus((w1-w0)*ms + (b1-b0)), l1 = -softplus((w0-w1)*ms + (b0-b1))
            res = small_pool.tile([P, 2], F32, tag="res")
            for col in range(2):
                ex = small_pool.tile([P, 1], F32, tag="sp", name=f"ex{col}")
                nc.scalar.activation(ex[:], ms[:], AF.Exp,
                                     bias=lin_sb[:, 2 * col + 1:2 * col + 2],
                                     scale=lin_sb[:, 2 * col:2 * col + 1])
                sp = small_pool.tile([P, 1], F32, tag="sp", name=f"sp{col}")
                nc.scalar.activation(sp[:], ex[:], AF.Ln, bias=1.0)
                nc.scalar.activation(res[:, col:col + 1], sp[:], AF.Copy,
                                     scale=-1.0)
            nc.sync.dma_start(out[j * P:(j + 1) * P, :], res[:])

    nc.compile()
    return nc


def _prep_weights(W_ih, W_hh, b_ih, b_hh, pos_table, lin_w, lin_b):
    bf = ml_dtypes.bfloat16
    # whh[t, p, (q, kk, c)] = W_hh[128*(15q+t)+c, 128kk+p]
    whh = np.ascontiguousarray(
        W_hh.reshape(4, KT, P, KT, P).transpose(1, 4, 0, 3, 2)
        .reshape(KT, P, 4 * KT * P)).astype(bf)
    # wie[p, (m, c)] = W_ih[128m+c, p]  (embedding half of W_ih)
    wie = np.ascontiguousarray(
        W_ih[:, :E].reshape(MT, P, E).transpose(2, 0, 1).reshape(P, MT * P)).astype(bf)
    # pos_proj[s, unit] = pos_table[s] @ W_ih[:, 128:].T + b_ih + b_hh
    pos_proj = pos_table @ W_ih[:, E:].T + (b_ih + b_hh)[None, :]
    pp = np.ascontiguousarray(
        pos_proj.reshape(SEQ, MT, P).transpose(2, 0, 1).reshape(P, SEQ * MT)
    ).astype(np.float32)
    w0, w1 = float(lin_w[0, 0]), float(lin_w[1, 0])
    b0, b1 = float(lin_b[0]), float(lin_b[1])
    lin = np.tile(np.array([[w1 - w0, b1 - b0, w0 - w1, b0 - b1]], np.float32),
                  (P, 1))
    return whh, wie, pp, lin


def kernel(input1, input2, emb_table, pos_table, W_ih, W_hh, b_ih, b_hh,
           lin_w, lin_b):
    global _COMPILED, LAST_RESULTS
    input1 = np.asarray(input1, np.int32)
    input2 = np.asarray(input2, np.int32)
    emb_table = np.ascontiguousarray(np.asarray(emb_table, np.float32))
    whh, wie, pp, lin = _prep_weights(
        np.asarray(W_ih, np.float32), np.asarray(W_hh, np.float32),
        np.asarray(b_ih, np.float32), np.asarray(b_hh, np.float32),
        np.asarray(pos_table, np.float32), np.asarray(lin_w, np.float32),
        np.asarray(lin_b, np.float32))

    if _COMPILED is None:
        _COMPILED = _build_program()
    nc = _COMPILED

    in_maps = []
    for c in range(NCORES):
        s1 = input1[c * BL:(c + 1) * BL]          # [512, 19]
        s2 = input2[c * BL:(c + 1) * BL]          # [512, 20]
        idx1 = np.ascontiguousarray(
            s1.reshape(JB, P, SEQ).transpose(1, 0, 2).reshape(P, JB * SEQ))
        idx2 = np.ascontiguousarray(
            s2.reshape(JB, P, N2).transpose(1, 0, 2).reshape(P, JB * N2))
        in_maps.append({
            "idx1": idx1, "idx2": idx2, "emb": emb_table,
            "whh": whh, "wie": wie, "pp": pp, "lin": lin,
        })

    res = run_bass_kernel_spmd(nc, in_maps, core_ids=list(range(NCORES)),
                               trace=TRACE)
    LAST_RESULTS = res
    return np.concatenate([res.results[c]["out"] for c in range(NCORES)], axis=0)


# revision 10
# speedup vs baseline: 2.7628x; 2.7628x over previous
"""Trainium2 Bass kernel for nn_LookupTablePosMy (embedding lookups + LSTM + windowed dot-product head).

Strategy: pure data-parallel over the batch (4096 -> 512 rows/core on 8 cores), no collectives.
Per core:
  - gather input1 embeddings (indirect DMA), PE-transpose to [emb, batch] bf16
  - LSTM over 19 steps: gates.T [7680, 512] computed in 60 [128,512] PSUM tiles,
    each accumulating 1 emb-projection chunk + 15 hidden chunks (bf16 matmuls),
    W_hh streamed from HBM each step in host-pre-transposed tile order,
    pos_table@W_ih.T + biases folded into the ACT bias operand.
  - head: gather input2 embeddings, transpose final h, 90 sliding-window dot
    products per sample via tensor_tensor_reduce (mult + chained max),
    log-softmax of 2 classes via Softplus.
"""

import sys

for _p in ("/opt/trn_rl_repo", "/opt/pypackages"):
    if _p not in sys.path:
        sys.path.append(_p)

import numpy as np
import ml_dtypes
from contextlib import ExitStack

import concourse.bass as bass
import concourse.bacc as bacc
import concourse.tile as tile
from concourse import mybir
from concourse.bass import IndirectOffsetOnAxis
from concourse.bass_utils import run_bass_kernel_spmd
from concourse.masks import make_identity

AF = mybir.ActivationFunctionType
ALU = mybir.AluOpType
F32 = mybir.dt.float32
BF16 = mybir.dt.bfloat16
I32 = mybir.dt.int32
FP8 = mybir.dt.float8e4

W_SCALE = 64.0            # fp8 W_hh scale (avoids e4m3 subnormals)
H_SCALE = 256.0           # fp8 h scale
GATE_DESCALE = 1.0 / (W_SCALE * H_SCALE)
UPAIR = 7                 # 7 DoubleRow k-pairs (hidden chunks 0..13), chunk 14 single

NCORES = 8
B = 4096
BL = B // NCORES          # 512 batch rows per core
P = 128
SEQ = 19
E = 128
H = 1920                  # hidden size
KT = H // P               # 15 hidden k-tiles
G = 4 * H                 # 7680 gate rows
MT = G // P               # 60 gate m-tiles
N2 = 20
NW = 18                   # windows
K5 = 5
D3 = 384                  # window dot width
JB = BL // P              # 4 batch tiles per core

TRACE = False             # test.py sets this for profiling runs
LAST_RESULTS = None       # BassKernelResults of last run (for test.py)

_COMPILED = None          # cached (nc,) built program


def _build_program(seq=SEQ, with_head=True, with_gathers=True):
    nc = bacc.Bacc("TRN2", target_bir_lowering=False, debug=False,
                   enable_asserts=False, num_devices=NCORES)

    # DRAM I/O (per-core shapes)
    idx1 = nc.dram_tensor("idx1", [P, JB * SEQ], I32, kind="ExternalInput").ap()
    idx2 = nc.dram_tensor("idx2", [P, JB * N2], I32, kind="ExternalInput").ap()
    emb = nc.dram_tensor("emb", [100000, E], F32, kind="ExternalInput").ap()
    whh = nc.dram_tensor("whh", [KT, P, 4 * (UPAIR * 2 + 1) * P], FP8, kind="ExternalInput").ap()
    wie = nc.dram_tensor("wie", [P, MT * P], BF16, kind="ExternalInput").ap()
    pp = nc.dram_tensor("pp", [P, SEQ * MT], F32, kind="ExternalInput").ap()
    lin = nc.dram_tensor("lin", [P, 4], F32, kind="ExternalInput").ap()
    out = nc.dram_tensor("out", [BL, 2], F32, kind="ExternalOutput").ap()

    with tile.TileContext(nc) as tc, ExitStack() as ctx:
        const_pool = ctx.enter_context(tc.tile_pool(name="const", bufs=1))
        w_pool = ctx.enter_context(tc.tile_pool(name="w", bufs=2))
        gather_pool = ctx.enter_context(tc.tile_pool(name="gat", bufs=12))
        gcast_pool = ctx.enter_context(tc.tile_pool(name="gc", bufs=8))
        e1t_pool = ctx.enter_context(tc.tile_pool(name="e1t", bufs=8))
        hp_pool = ctx.enter_context(tc.tile_pool(name="hp", bufs=16))
        hl_pool = ctx.enter_context(tc.tile_pool(name="hl", bufs=3))
        c_pool = ctx.enter_context(tc.tile_pool(name="c", bufs=KT))
        gact_pool = ctx.enter_context(tc.tile_pool(name="gact", bufs=8))
        emb2_pool = ctx.enter_context(tc.tile_pool(name="emb2", bufs=JB))
        hT_pool = ctx.enter_context(tc.tile_pool(name="hT", bufs=JB))
        head_pool = ctx.enter_context(tc.tile_pool(name="hsc", bufs=4))
        small_pool = ctx.enter_context(tc.tile_pool(name="small", bufs=16))
        mm_psum = ctx.enter_context(tc.tile_pool(name="mmps", bufs=6, space="PSUM"))
        tr_psum = ctx.enter_context(tc.tile_pool(name="trps", bufs=2, space="PSUM"))

        # constants
        ident = const_pool.tile([P, P], BF16)
        make_identity(nc, ident[:])
        idx1_sb = const_pool.tile([P, JB * SEQ], I32)
        nc.sync.dma_start(idx1_sb[:], idx1[:])
        idx2_sb = const_pool.tile([P, JB * N2], I32)
        nc.sync.dma_start(idx2_sb[:], idx2[:])
        pp_sb = const_pool.tile([P, SEQ * MT], F32)
        nc.sync.dma_start(pp_sb[:], pp[:])
        wie_sb = const_pool.tile([P, MT * P], BF16)
        nc.sync.dma_start(wie_sb[:], wie[:])
        lin_sb = const_pool.tile([P, 4], F32)
        nc.sync.dma_start(lin_sb[:], lin[:])

        # ---- input1 embedding gathers -> transpose -> e1t[t] [128(emb), 512(batch)] bf16
        e1t = []
        for t in range(seq):
            et = e1t_pool.tile([P, BL], BF16, tag="e1t")
            for j in range(JB):
                if with_gathers:
                    gt = gather_pool.tile([P, E], F32, tag="gat")
                    nc.gpsimd.indirect_dma_start(
                        out=gt[:], out_offset=None, in_=emb[:],
                        in_offset=IndirectOffsetOnAxis(
                            ap=idx1_sb[:, j * SEQ + t: j * SEQ + t + 1], axis=0))
                    gc = gcast_pool.tile([P, E], BF16, tag="gc")
                    nc.vector.tensor_copy(gc[:], gt[:])
                    tp = tr_psum.tile([P, P], BF16, tag="trps")
                    nc.tensor.transpose(tp[:], gc[:], ident[:])
                    nc.vector.tensor_copy(et[:, j * P:(j + 1) * P], tp[:])
                else:
                    nc.vector.memset(et[:, j * P:(j + 1) * P], 0.01)
            e1t.append(et)

        # ---- input2 embedding gathers -> emb2[j] [128(batch), 2560(emb)] bf16
        emb2 = []
        for j in range(JB):
            e2 = emb2_pool.tile([P, N2 * E], BF16, tag="emb2")
            for n in range(N2):
                if with_gathers and with_head:
                    gt = gather_pool.tile([P, E], F32, tag="gat")
                    nc.gpsimd.indirect_dma_start(
                        out=gt[:], out_offset=None, in_=emb[:],
                        in_offset=IndirectOffsetOnAxis(
                            ap=idx2_sb[:, j * N2 + n: j * N2 + n + 1], axis=0))
                    nc.vector.tensor_copy(e2[:, n * E:(n + 1) * E], gt[:])
                else:
                    nc.vector.memset(e2[:, n * E:(n + 1) * E], 0.01)
            emb2.append(e2)

        # ---- LSTM over 19 steps
        c_tiles = [c_pool.tile([P, BL], F32, tag="c", name=f"c{_k}") for _k in range(KT)]
        h_prev_pairs, h_prev_last = None, None
        for s in range(seq):
            h_pairs = [hp_pool.tile([P, 2 * BL], FP8, tag="hp", name=f"hp{s}_{u}")
                       for u in range(UPAIR)]
            h_last = hl_pool.tile([P, BL], FP8, tag="hl", name=f"hl{s}")
            for t in range(KT):
                if s > 0:
                    wt = w_pool.tile([P, 4 * (UPAIR * 2 + 1) * P], FP8, tag="w")
                    nc.sync.dma_start(wt[:], whh[t])
                acts = []
                for q in range(4):
                    m = q * KT + t
                    ps = mm_psum.tile([P, BL], F32, tag="mmps")
                    nc.tensor.matmul(ps[:], wie_sb[:, m * P:(m + 1) * P], e1t[s][:],
                                     start=True, stop=(s == 0))
                    if s > 0:
                        for u in range(UPAIR):
                            base = (q * UPAIR + u) * 2 * P
                            nc.tensor.matmul(
                                ps[:],
                                wt[:, base:base + 2 * P].rearrange(
                                    "p (o c) -> p o c", o=2),
                                h_prev_pairs[u][:].rearrange(
                                    "p (o n) -> p o n", o=2),
                                start=False, stop=False,
                                perf_mode=mybir.MatmulPerfMode.DoubleRow)
                        tbase = 4 * UPAIR * 2 * P + q * P
                        nc.tensor.matmul(ps[:], wt[:, tbase:tbase + P],
                                         h_prev_last[:], start=False, stop=True)
                    func = AF.Tanh if q == 2 else AF.Sigmoid
                    dst = gact_pool.tile([P, BL], F32, tag="gact")
                    nc.scalar.activation(dst[:], ps[:], func, scale=GATE_DESCALE,
                                         bias=pp_sb[:, s * MT + m: s * MT + m + 1])
                    acts.append(dst)
                i_, f_, g_, o_ = acts
                ct = c_tiles[t]
                if s == 0:
                    nc.vector.tensor_tensor(ct[:], i_[:], g_[:], op=ALU.mult)
                else:
                    nc.vector.tensor_tensor(i_[:], i_[:], g_[:], op=ALU.mult)
                    nc.vector.tensor_tensor(ct[:], f_[:], ct[:], op=ALU.mult)
                    nc.vector.tensor_tensor(ct[:], ct[:], i_[:], op=ALU.add)
                nc.scalar.activation(g_[:], ct[:], AF.Tanh)
                if t < 2 * UPAIR:
                    hdst = h_pairs[t // 2][:, (t % 2) * BL:(t % 2 + 1) * BL]
                else:
                    hdst = h_last[:]
                nc.vector.scalar_tensor_tensor(hdst, o_[:], H_SCALE, g_[:],
                                               op0=ALU.mult, op1=ALU.mult)
            h_prev_pairs, h_prev_last = h_pairs, h_last

        # ---- head
        for j in range(JB):
            hT = hT_pool.tile([P, H], BF16, tag="hT")
            for k in range(KT):
                if k < 2 * UPAIR:
                    hsrc = h_prev_pairs[k // 2][:, (k % 2) * BL + j * P:
                                                (k % 2) * BL + (j + 1) * P]
                else:
                    hsrc = h_prev_last[:, j * P:(j + 1) * P]
                hb = gcast_pool.tile([P, P], BF16, tag="gc", name=f"hb{j}_{k}")
                nc.vector.tensor_copy(hb[:], hsrc)
                tp = tr_psum.tile([P, P], BF16, tag="trps")
                nc.tensor.transpose(tp[:], hb[:], ident[:])
                nc.vector.tensor_scalar_mul(hT[:, k * P:(k + 1) * P], tp[:],
                                            1.0 / H_SCALE)
            ms = small_pool.tile([P, 1], F32, tag="ms")
            rs = small_pool.tile([P, K5 * NW], F32, tag="rs")
            for n in range(NW):
                win = emb2[j][:, n * E:n * E + D3]
                win3 = win.rearrange("p (o d) -> p o d", o=1).to_broadcast(
                    [P, K5, D3])
                prod = head_pool.tile([P, H], BF16, tag="hsc")
                nc.vector.tensor_tensor(prod[:], hT[:], win3, op=ALU.mult)
                nc.vector.tensor_reduce(
                    out=rs[:, n * K5:(n + 1) * K5],
                    in_=prod[:].rearrange("p (k d) -> p k d", k=K5),
                    axis=mybir.AxisListType.X, op=ALU.add)
            nc.vector.tensor_reduce(out=ms[:, 0:1], in_=rs[:],
                                    axis=mybir.AxisListType.X, op=ALU.max)
            # logits = log_softmax([w0*ms+b0, w1*ms+b1]):
            #   l0 = -softplus((w1-w0)*ms + (b1-b0)), l1 = -softplus((w0-w1)*ms + (b0-b1))
            res = small_pool.tile([P, 2], F32, tag="res")
            for col in range(2):
                ex = small_pool.tile([P, 1], F32, tag="sp", name=f"ex{col}")
                nc.scalar.activation(ex[:], ms[:], AF.Exp,
                                     bias=lin_sb[:, 2 * col + 1:2 * col + 2],
                                     scale=lin_sb[:, 2 * col:2 * col + 1])
                sp = small_pool.tile([P, 1], F32, tag="sp", name=f"sp{col}")
                nc.scalar.activation(sp[:], ex[:], AF.Ln, bias=1.0)
                nc.scalar.activation(res[:, col:col + 1], sp[:], AF.Copy,
                                     scale=-1.0)
            nc.sync.dma_start(out[j * P:(j + 1) * P, :], res[:])

    nc.compile()
    return nc


def _prep_weights(W_ih, W_hh, b_ih, b_hh, pos_table, lin_w, lin_b):
    bf = ml_dtypes.bfloat16
    fp8 = mybir.dt.np(FP8)
    # fp8 DoubleRow layout, scaled by W_SCALE:
    #   block A[t, p, q, u, o, c] = W_hh[128*(15q+t)+c, 128*(2u+o)+p]
    #   block B[t, p, q, c]       = W_hh[128*(15q+t)+c, 128*14+p]
    Ws = (W_hh * W_SCALE).astype(np.float32)
    W4 = Ws.reshape(4, KT, P, H)                      # [q, t, c, hid]
    A = (W4[:, :, :, :2 * UPAIR * P]
         .reshape(4, KT, P, UPAIR, 2, P)              # [q, t, c, u, o, p]
         .transpose(1, 5, 0, 3, 4, 2)                 # [t, p, q, u, o, c]
         .reshape(KT, P, 4 * UPAIR * 2 * P))
    Bt = (W4[:, :, :, 2 * UPAIR * P:]                 # [q, t, c, p]
          .transpose(1, 3, 0, 2)                      # [t, p, q, c]
          .reshape(KT, P, 4 * P))
    whh = np.ascontiguousarray(
        np.concatenate([A, Bt], axis=2)).astype(fp8)
    # wie[p, (m, c)] = W_ih[128m+c, p] * W_SCALE*H_SCALE  (embedding half)
    wie = np.ascontiguousarray(
        (W_ih[:, :E] * (W_SCALE * H_SCALE))
        .reshape(MT, P, E).transpose(2, 0, 1).reshape(P, MT * P)).astype(bf)
    # pos_proj[s, unit] = pos_table[s] @ W_ih[:, 128:].T + b_ih + b_hh
    pos_proj = pos_table @ W_ih[:, E:].T + (b_ih + b_hh)[None, :]
    pp = np.ascontiguousarray(
        pos_proj.reshape(SEQ, MT, P).transpose(2, 0, 1).reshape(P, SEQ * MT)
    ).astype(np.float32)
    w0, w1 = float(lin_w[0, 0]), float(lin_w[1, 0])
    b0, b1 = float(lin_b[0]), float(lin_b[1])
    lin = np.tile(np.array([[w1 - w0, b1 - b0, w0 - w1, b0 - b1]], np.float32),
                  (P, 1))
    return whh, wie, pp, lin


def kernel(input1, input2, emb_table, pos_table, W_ih, W_hh, b_ih, b_hh,
           lin_w, lin_b):
    global _COMPILED, LAST_RESULTS
    input1 = np.asarray(input1, np.int32)
    input2 = np.asarray(input2, np.int32)
    emb_table = np.ascontiguousarray(np.asarray(emb_table, np.float32))
    whh, wie, pp, lin = _prep_weights(
        np.asarray(W_ih, np.float32), np.asarray(W_hh, np.float32),
        np.asarray(b_ih, np.float32), np.asarray(b_hh, np.float32),
        np.asarray(pos_table, np.float32), np.asarray(lin_w, np.float32),
        np.asarray(lin_b, np.float32))

    if _COMPILED is None:
        _COMPILED = _build_program()
    nc = _COMPILED

    in_maps = []
    for c in range(NCORES):
        s1 = input1[c * BL:(c + 1) * BL]          # [512, 19]
        s2 = input2[c * BL:(c + 1) * BL]          # [512, 20]
        idx1 = np.ascontiguousarray(
            s1.reshape(JB, P, SEQ).transpose(1, 0, 2).reshape(P, JB * SEQ))
        idx2 = np.ascontiguousarray(
            s2.reshape(JB, P, N2).transpose(1, 0, 2).reshape(P, JB * N2))
        in_maps.append({
            "idx1": idx1, "idx2": idx2, "emb": emb_table,
            "whh": whh, "wie": wie, "pp": pp, "lin": lin,
        })

    res = run_bass_kernel_spmd(nc, in_maps, core_ids=list(range(NCORES)),
                               trace=TRACE)
    LAST_RESULTS = res
    return np.concatenate([res.results[c]["out"] for c in range(NCORES)], axis=0)
